# revision 5
# baseline (speedup 1.0000x reference)
"""Bag-of-words histogram kernel for Trainium2 (Bass/Tile), 8-core data-parallel.

Problem: docs [256, 2048] int32 token ids in [0, 32000) ->
         hist [256, 32000] fp32, hist[b, v] = count(docs[b, :] == v) / 2048.

Algorithm (per core, 32 rows):
  Factor each token t = 256*hi + lo (hi < 125, lo < 256). Then
    hist[b, hi, lo] = sum_s onehot_hi[s, hi] * onehot_lo[s, lo]
  computed as fp8e4 one-hot outer products on the PE, accumulated in PSUM
  over 16 k-tiles of 128 tokens per row.

  One-hot construction trick: the fp8e4 one-hot byte vectors are built as
  *uint16 pair words* with a single fused DVE tensor_scalar:
      out_u16[p, j] = (j == byte_pos>>1) * v    where v places the fp8 value
  at the low or high byte of the pair (byte_pos & 1). This halves the DVE
  element count vs bf16 one-hots (u16 words cover 2 fp8 bytes) and runs in
  the 4x DVE perf mode (2-byte dtype, SBUF, tensor_scalar).
  The one-hot values are 2^-5 (hi side) and 2^-6 (lo side), so PSUM
  accumulates hist/2048 exactly and no separate scaling pass is needed.

  A fraction of hi one-hots is built on the scalar (ACT) engine instead
  (|d| then relu(2^-5 - 2^-5 |d|), written directly as fp8e4 bytes) to
  offload the DVE bottleneck. PSUM->SBUF copies run on ACT; DMA writes the
  [125, 256] fp32 tile per row straight to HBM.

Sharding: batch axis split 8 ways (32 rows per core), no communication.
"""

import sys

import numpy as np

for _p in ("/opt/trn_rl_repo",):
    if _p not in sys.path:
        sys.path.append(_p)

BATCH = 256
SEQ = 2048
VOCAB = 32000
N_CORES = 8
ROWS = BATCH // N_CORES  # 32 rows per core
P = 128
KT = SEQ // P            # 16 k-tiles per row
GR = 8                   # rows per input-DMA group
NLO = 256                # lo = t & 255
NHI = 128                # hi = t >> 8 < 125, padded to 128

# k-tiles whose hi one-hot is built on the ACT engine (per row).
ACT_KS = frozenset({2, 5, 8, 11})


def _build_nc():
    from contextlib import ExitStack

    from concourse import bacc, bass, mybir
    from concourse.tile import TileContext

    nc = bacc.Bacc()
    docs = nc.dram_tensor("docs", [ROWS, SEQ], mybir.dt.int32, kind="ExternalInput")
    hist = nc.dram_tensor("hist", [ROWS, VOCAB], mybir.dt.float32, kind="ExternalOutput")

    f32 = mybir.dt.float32
    bf16 = mybir.dt.bfloat16
    u16 = mybir.dt.uint16
    fp8 = mybir.dt.float8e4
    i32 = mybir.dt.int32
    Alu = mybir.AluOpType
    Act = mybir.ActivationFunctionType

    V_HI = 2.0 ** -5   # fp8e4 0x10
    V_LO = 2.0 ** -6   # fp8e4 0x08
    # uint16 pair-words placing the fp8 byte at the low/high byte position
    VHI_EVEN, VHI_ODD = 0x0010, 0x1000
    VLO_EVEN, VLO_ODD = 0x0008, 0x0800

    with TileContext(nc) as tc, ExitStack() as ctx:
        const_tp = ctx.enter_context(tc.tile_pool(name="const", bufs=1))
        tok_tp = ctx.enter_context(tc.tile_pool(name="tok", bufs=4))
        sc_tp = ctx.enter_context(tc.tile_pool(name="sc", bufs=4))
        oh_tp = ctx.enter_context(tc.tile_pool(name="oh", bufs=16))
        res_tp = ctx.enter_context(tc.tile_pool(name="res", bufs=4))
        psum_tp = ctx.enter_context(tc.tile_pool(name="psum", bufs=8, space="PSUM"))

        # iota constants (value = column index on every partition)
        iota_lo = const_tp.tile([P, NLO // 2], u16)   # 0..127 (lo pair index)
        nc.gpsimd.iota(iota_lo[:], [[1, NLO // 2]], channel_multiplier=0)
        iota_hi = const_tp.tile([P, NHI // 2], u16)   # 0..63 (hi pair index)
        nc.gpsimd.iota(iota_hi[:], [[1, NHI // 2]], channel_multiplier=0)
        iota_hib = const_tp.tile([P, NHI], bf16)      # 0..127 for the ACT path
        nc.gpsimd.iota(iota_hib[:], [[1, NHI]], channel_multiplier=0,
                       allow_small_or_imprecise_dtypes=True)
        vhi_bias = const_tp.tile([P, 1], f32)         # ACT relu bias constant
        nc.gpsimd.memset(vhi_bias[:], V_HI)

        for g in range(ROWS // GR):
            # Load GR rows; partition p holds tokens [16p, 16p+16) of each row
            # (any within-row permutation is histogram-invariant).
            tok = tok_tp.tile([P, GR, KT], i32)
            src = bass.AP(docs, g * GR * SEQ, [[16, P], [SEQ, GR], [1, KT]])
            nc.sync.dma_start(out=tok[:], in_=src)

            # Per-token compare targets / pair values, as fp32 scalar planes.
            jlo_i = sc_tp.tile([P, GR, KT], i32, tag="jloi")
            nc.vector.tensor_scalar(out=jlo_i[:], in0=tok[:], scalar1=1,
                                    scalar2=127, op0=Alu.logical_shift_right,
                                    op1=Alu.bitwise_and)
            jlo_f = sc_tp.tile([P, GR, KT], f32, tag="jlof")
            nc.vector.tensor_scalar(out=jlo_f[:], in0=jlo_i[:], scalar1=1.0,
                                    scalar2=None, op0=Alu.mult)

            b0 = sc_tp.tile([P, GR, KT], i32, tag="b0")
            nc.vector.tensor_scalar(out=b0[:], in0=tok[:], scalar1=1,
                                    scalar2=None, op0=Alu.bitwise_and)
            vlo_f = sc_tp.tile([P, GR, KT], f32, tag="vlof")
            nc.vector.tensor_scalar(out=vlo_f[:], in0=b0[:],
                                    scalar1=float(VLO_ODD - VLO_EVEN),
                                    scalar2=float(VLO_EVEN),
                                    op0=Alu.mult, op1=Alu.add)

            jhi_i = sc_tp.tile([P, GR, KT], i32, tag="jhii")
            nc.vector.tensor_scalar(out=jhi_i[:], in0=tok[:], scalar1=9,
                                    scalar2=None, op0=Alu.logical_shift_right)
            jhi_f = sc_tp.tile([P, GR, KT], f32, tag="jhif")
            nc.vector.tensor_scalar(out=jhi_f[:], in0=jhi_i[:], scalar1=1.0,
                                    scalar2=None, op0=Alu.mult)

            b8 = sc_tp.tile([P, GR, KT], i32, tag="b8")
            nc.vector.tensor_scalar(out=b8[:], in0=tok[:], scalar1=8,
                                    scalar2=1, op0=Alu.logical_shift_right,
                                    op1=Alu.bitwise_and)
            vhi_f = sc_tp.tile([P, GR, KT], f32, tag="vhif")
            nc.vector.tensor_scalar(out=vhi_f[:], in0=b8[:],
                                    scalar1=float(VHI_ODD - VHI_EVEN),
                                    scalar2=float(VHI_EVEN),
                                    op0=Alu.mult, op1=Alu.add)

            if ACT_KS:
                hi_i = sc_tp.tile([P, GR, KT], i32, tag="hii")
                nc.vector.tensor_scalar(out=hi_i[:], in0=tok[:], scalar1=8,
                                        scalar2=None,
                                        op0=Alu.logical_shift_right)
                nhi_f = sc_tp.tile([P, GR, KT], f32, tag="nhif")
                nc.vector.tensor_scalar(out=nhi_f[:], in0=hi_i[:], scalar1=-1.0,
                                        scalar2=None, op0=Alu.mult)

            for rl in range(GR):
                r = g * GR + rl
                ps = psum_tp.tile([P, NLO], f32)
                for k in range(KT):
                    oh_hi = oh_tp.tile([P, NHI], fp8, tag="ohhi")
                    oh_lo = oh_tp.tile([P, NLO], fp8, tag="ohlo")
                    if k in ACT_KS:
                        # |c - hi| then relu(2^-5 - 2^-5 d): exact fp8 one-hot
                        d = oh_tp.tile([P, NHI], bf16, tag="dabs")
                        nc.scalar.activation(
                            out=d[:], in_=iota_hib[:], func=Act.Abs,
                            bias=nhi_f[:, rl, k:k + 1], scale=1.0)
                        nc.scalar.activation(
                            out=oh_hi[:], in_=d[:], func=Act.Relu,
                            bias=vhi_bias[:], scale=-V_HI)
                    else:
                        nc.vector.tensor_scalar(
                            out=oh_hi[:].bitcast(u16), in0=iota_hi[:],
                            scalar1=jhi_f[:, rl, k:k + 1],
                            scalar2=vhi_f[:, rl, k:k + 1],
                            op0=Alu.is_equal, op1=Alu.mult)
                    nc.vector.tensor_scalar(
                        out=oh_lo[:].bitcast(u16), in0=iota_lo[:],
                        scalar1=jlo_f[:, rl, k:k + 1],
                        scalar2=vlo_f[:, rl, k:k + 1],
                        op0=Alu.is_equal, op1=Alu.mult)
                    nc.tensor.matmul(out=ps[:], lhsT=oh_hi[:], rhs=oh_lo[:],
                                     start=(k == 0), stop=(k == KT - 1))

                res = res_tp.tile([P, NLO], f32)
                nc.scalar.copy(out=res[:], in_=ps[:])
                nc.sync.dma_start(
                    out=hist[r].rearrange("(h l) -> h l", l=NLO),
                    in_=res[:VOCAB // NLO, :])
    nc.compile()
    return nc


_NC_CACHE = None


def _get_nc():
    global _NC_CACHE
    if _NC_CACHE is None:
        _NC_CACHE = _build_nc()
    return _NC_CACHE


def run_sharded(docs: np.ndarray, trace: bool = False):
    """Run the 8-core SPMD kernel. Returns (full_output, BassKernelResults)."""
    from concourse.bass_utils import run_bass_kernel_spmd

    docs = np.ascontiguousarray(np.asarray(docs, dtype=np.int32))
    assert docs.shape == (BATCH, SEQ), docs.shape
    shards = docs.reshape(N_CORES, ROWS, SEQ)
    in_maps = [{"docs": shards[i]} for i in range(N_CORES)]
    res = run_bass_kernel_spmd(_get_nc(), in_maps, core_ids=list(range(N_CORES)),
                               trace=trace)
    out = np.concatenate([res.results[i]["hist"] for i in range(N_CORES)], axis=0)
    return out, res


def kernel(docs: np.ndarray) -> np.ndarray:
    out, _ = run_sharded(docs, trace=False)
    return out
